# revision 14
# baseline (speedup 1.0000x reference)
"""Trainium2 Bass kernel for the DPLR state-space model (S4-style FFT conv).

Strategy (no collectives; 8 cores = 4 batches x 2 channel-halves):
  - Host precomputes (fp64-exact) the chunked-SSM operators from the tiny
    [D,N] SSM parameters: Toeplitz intra-chunk conv matrices S_T (with the
    skip connection folded into lag 0), state-gather weights W_state, the
    state-broadcast matrices Qp, and chunk-decay factors A^T.
  - Device per core (b = core//2, h = core%2, 512 local channels):
      proj1: x_in = x[b] @ W_in[half]^T  (fp16 matmuls, chunk layout)
      conv:  per-channel intra-chunk matmul (V_d stationary [128,16],
             Toeplitz moving [128,128]) + chunk-state recurrence via
             tensor_tensor_scan on DVE + inter-chunk correction matmuls
             (K=16, accumulated into the same PSUM strips)
      proj2: partial out_T = W_out[:, half-cols] @ y   (fp16)
  - Host sums the two partial outputs per batch (exact linearity of W_out).

Chunking: L=2048 split into C=16 chunks of T=128.  All matmul operands are
fp16 (PSUM accumulation in fp32); expected end-to-end relative error ~1e-3.
"""

import numpy as np

import concourse.bass as bass
import concourse.bacc as bacc
import concourse.mybir as mybir
from concourse.tile import TileContext
from concourse.bass_utils import run_bass_kernel_spmd

# Problem shape (hardcoded per contract)
B, L, D, N = 4, 2048, 1024, 16
T = 128          # chunk length == SBUF partitions
CN = L // T      # 16 chunks
DL = D // 2      # 512 local channels per core
NG = DL // 4     # 128 groups of 4 channels (one PSUM bank strip set each)

DT = mybir.dt.float16
F32 = mybir.dt.float32


# --------------------------------------------------------------------------
# Device program (identical on all 8 cores; SPMD over per-core data)
# --------------------------------------------------------------------------

def build_nc():
    nc = bacc.Bacc()

    xT = nc.declare_dram_parameter("xT", [128, 8 * 2048], DT, isOutput=False)
    winT = nc.declare_dram_parameter("winT", [128, 8 * 512], DT, isOutput=False)
    woutT = nc.declare_dram_parameter("woutT", [128, 4 * 1024], DT, isOutput=False)
    s_t = nc.declare_dram_parameter("s_t", [128, DL * 128], DT, isOutput=False)
    wstate = nc.declare_dram_parameter("wstate", [128, DL * 16], DT, isOutput=False)
    qp = nc.declare_dram_parameter("qp", [128, NG * 128], DT, isOutput=False)
    d0 = nc.declare_dram_parameter("d0", [128, NG * 32], DT, isOutput=False)
    outT = nc.declare_dram_parameter("outT", [128, 8 * 2048], DT, isOutput=True)
    # DRAM bounce buffer for the conv-output partition regroup:
    # yscr[k, s*2048 + c*128 + t] = y[dl = s*128 + k, l = c*128 + t]
    yscr = nc.dram_tensor("yscr", [128, 4 * 2048], DT)

    with TileContext(nc) as tc:
        with (
            tc.tile_pool(name="cpool", bufs=1) as cpool,
            tc.tile_pool(name="spool", bufs=2) as spool,
            tc.tile_pool(name="ypool", bufs=2) as ypool,
            tc.tile_pool(name="opool", bufs=2) as opool,
        ):
            # persistent SBUF tensors (straight contiguous loads)
            xT_sb = cpool.tile([128, 8 * 2048], DT, name="xT_sb")
            winT_sb = cpool.tile([128, 8 * 512], DT, name="winT_sb")
            woutT_sb = cpool.tile([128, 4 * 1024], DT, name="woutT_sb")
            wstate_sb = cpool.tile([128, DL * 16], DT, name="wstate_sb")
            qp_sb = cpool.tile([128, NG * 128], DT, name="qp_sb")
            d0_sb = cpool.tile([128, NG * 32], DT, name="d0_sb")
            nc.sync.dma_start(out=xT_sb[:, :], in_=xT[:, :])
            nc.sync.dma_start(out=winT_sb[:, :], in_=winT[:, :])
            nc.sync.dma_start(out=woutT_sb[:, :], in_=woutT[:, :])
            nc.sync.dma_start(out=wstate_sb[:, :], in_=wstate[:, :])
            nc.sync.dma_start(out=qp_sb[:, :], in_=qp[:, :])
            nc.sync.dma_start(out=d0_sb[:, :], in_=d0[:, :])

            # x_in in chunk layout: xall[j, c*512 + dl]
            xall = cpool.tile([128, CN * DL], DT, name="xall")
            # chunk-state scan buffers: partition 32*s + n, free k*32 + q
            scanbuf = cpool.tile([128, NG * 32], DT, name="scanbuf")
            scanout = cpool.tile([128, NG * 32], DT, name="scanout")
            # conv output: y_sb[k, s*2048 + c*128 + t] = y[dl = s*128+k, l]
            y_sb = cpool.tile([128, 4 * 2048], DT, name="y_sb")

            nc.vector.memset(scanbuf[:, :], 0.0)

            # V_d access pattern helper: [j=128, c=16] strided view of xall
            xall_v = xall.rearrange("p (c d) -> p d c", d=DL)

            # ---- proj1: x_in[l, dl] = sum_f x[l,f] W_in[dl,f]
            with tc.tile_pool(name="pp1", bufs=4, space="PSUM") as pp1:
                for lt in range(CN):
                    ps1 = pp1.tile([128, 512], F32, tag="ps1", name="ps1")
                    for ft in range(8):
                        nc.tensor.matmul(
                            ps1[:, :],
                            lhsT=xT_sb[:, ft * 2048 + lt * 128: ft * 2048 + (lt + 1) * 128],
                            rhs=winT_sb[:, ft * 512:(ft + 1) * 512],
                            start=(ft == 0), stop=(ft == 7),
                        )
                    nc.vector.tensor_copy(xall[:, lt * 512:(lt + 1) * 512], ps1[:, :])

            # ---- pass A: u[n, c] per channel -> scanbuf
            with tc.tile_pool(name="ppA", bufs=8, space="PSUM") as ppA:
                for k in range(NG):
                    psA = ppA.tile([128, 16], F32, tag="psA", name="psA")
                    nc.vector.memset(psA[:, :], 0.0)
                    for s in range(4):
                        e = k * 4 + s          # stream-order channel index
                        dl = s * 128 + k       # local channel
                        nc.tensor.matmul(
                            psA[32 * s:32 * s + 16, :],
                            lhsT=wstate_sb[:, e * 16:(e + 1) * 16],
                            rhs=xall_v[:, dl, :],
                            start=True, stop=True,
                            tile_position=(0, 32 * s),
                        )
                    nc.vector.tensor_copy(scanbuf[:, k * 32 + 1:k * 32 + 17], psA[:, :])

            # ---- chunk-state scan: P[c] = aT*P[c-1] + u[c-1] (reset per group)
            nc.vector.tensor_tensor_scan(
                out=scanout[:, :], data0=d0_sb[:, :], data1=scanbuf[:, :],
                initial=0.0, op0=mybir.AluOpType.mult, op1=mybir.AluOpType.add,
            )

            # ---- pass B: y = S_T^T V (intra) + Qp^T P (inter)
            with tc.tile_pool(name="ppB", bufs=8, space="PSUM") as ppB:
                for kb in range(16):           # 8-group blocks (32 channels)
                    s_blk = spool.tile([128, 32 * 128], DT, tag="s_blk", name="s_blk")
                    nc.sync.dma_start(
                        out=s_blk[:, :], in_=s_t[:, kb * 4096:(kb + 1) * 4096])
                    yst = ypool.tile([128, 8 * 128], DT, tag="yst", name="yst")
                    for kk in range(8):
                        k = kb * 8 + kk
                        psB = ppB.tile([128, 128], F32, tag="psB", name="psB")
                        nc.vector.memset(psB[:, :], 0.0)
                        for s in range(4):
                            eb = kk * 4 + s    # channel index within block
                            dl = s * 128 + k
                            nc.tensor.matmul(
                                psB[32 * s:32 * s + 16, :],
                                lhsT=xall_v[:, dl, :],
                                rhs=s_blk[:, eb * 128:(eb + 1) * 128],
                                start=True, stop=False,
                                tile_position=(0, 32 * s),
                            )
                            nc.tensor.matmul(
                                psB[32 * s:32 * s + 16, :],
                                lhsT=scanout[32 * s:32 * s + 16, k * 32:k * 32 + 16],
                                rhs=qp_sb[32 * s:32 * s + 16, k * 128:(k + 1) * 128],
                                start=False, stop=True,
                                tile_position=(32 * s, 32 * s),
                            )
                        nc.vector.tensor_copy(yst[:, kk * 128:(kk + 1) * 128], psB[:, :])
                    # regroup: yst[(32s+c), kk*128+t] -> yscr[kb*8+kk, s*2048+c*128+t]
                    # (bounced through DRAM: SBUF APs must be partition-major)
                    for s in range(4):
                        src = yst[32 * s:32 * s + 16, :].rearrange(
                            "c (kk t) -> c kk t", t=128)
                        dst = yscr[kb * 8:kb * 8 + 8, s * 2048:(s + 1) * 2048].rearrange(
                            "kk (c t) -> c kk t", t=128)
                        nc.sync.dma_start(out=dst, in_=src)
                # gather the regrouped conv output back into SBUF
                nc.sync.dma_start(out=y_sb[:, :], in_=yscr[:, :])

            # ---- proj2: out_T[e, l] = sum_dl W_out[e, dl] y[dl, l] (partial)
            with tc.tile_pool(name="pp2", bufs=8, space="PSUM") as pp2:
                for m in range(8):
                    ost = opool.tile([128, 2048], DT, tag="ost", name="ost")
                    for lc in range(4):
                        ps2 = pp2.tile([128, 512], F32, tag="ps2", name="ps2")
                        for kt in range(4):
                            nc.tensor.matmul(
                                ps2[:, :],
                                lhsT=woutT_sb[:, kt * 1024 + m * 128: kt * 1024 + (m + 1) * 128],
                                rhs=y_sb[:, kt * 2048 + lc * 512: kt * 2048 + (lc + 1) * 512],
                                start=(kt == 0), stop=(kt == 3),
                            )
                        nc.vector.tensor_copy(ost[:, lc * 512:(lc + 1) * 512], ps2[:, :])
                    nc.sync.dma_start(
                        out=outT[:, m * 2048:(m + 1) * 2048], in_=ost[:, :])

    nc.finalize()
    return nc


# --------------------------------------------------------------------------
# Host-side operator precompute (fp64-exact) and data formatting
# --------------------------------------------------------------------------

def _ssm_operators(A_log, B_ssm, C_ssm, dt_log, D_ssm):
    """Full-D chunked-SSM operators, fp64."""
    A_log = A_log.astype(np.float64)
    B_ssm = B_ssm.astype(np.float64)
    C_ssm = C_ssm.astype(np.float64)
    dt_log = dt_log.astype(np.float64)
    D_ssm = D_ssm.astype(np.float64)

    A_diag = -np.exp(A_log)                       # [D, N]
    dt = np.exp(dt_log)[:, None]                  # [D, 1]
    logA = dt * A_diag                            # log(A_bar), exact
    A_bar = np.exp(logA)
    B_bar = (A_bar - 1.0) / A_diag * B_ssm
    CB = C_ssm * B_bar                            # [D, N]

    m = np.arange(T)
    A_pow = np.exp(logA[:, None, :] * m[None, :, None])       # [D, T, N]
    K = np.einsum("dn,dmn->dm", CB, A_pow)                    # [D, T]
    K[:, 0] += D_ssm                              # skip connection at lag 0

    # S_T[d, j, t] = K[d, t-j] for t >= j else 0
    idx = m[None, :] - m[:, None]                 # [j, t]
    Kp = np.concatenate([np.zeros((D, T)), K], axis=1)
    S_T = Kp[:, idx + T]                          # [D, T, T]

    W_state = np.exp(logA[:, None, :] * (T - 1 - m)[None, :, None])   # [D, T, N]
    Qp = CB[:, :, None] * np.exp(logA[:, :, None] * (m + 1)[None, None, :])  # [D,N,T]
    aT = np.exp(logA * T)                         # [D, N]
    return S_T, W_state, Qp, aT


def _half_arrays(S_T, W_state, Qp, aT, h):
    """Format one channel-half's operator arrays into device layouts (fp16)."""
    sl = slice(h * DL, (h + 1) * DL)
    S_l, W_l, Q_l, a_l = S_T[sl], W_state[sl], Qp[sl], aT[sl]

    # stream order e -> dl(e) = (e%4)*128 + e//4
    e = np.arange(DL)
    perm = (e % 4) * 128 + e // 4

    s_t_h = np.ascontiguousarray(
        S_l[perm].transpose(1, 0, 2).reshape(128, DL * 128)).astype(np.float16)
    wstate_h = np.ascontiguousarray(
        W_l[perm].transpose(1, 0, 2).reshape(128, DL * 16)).astype(np.float16)

    # qp[32s+n, k*128+t] = Qp[dl=s*128+k, n, t]
    q_r = Q_l.reshape(4, 128, N, T)               # [s, k, n, t]
    q_full = np.zeros((4, 32, 128, 128))
    q_full[:, :N] = q_r.transpose(0, 2, 1, 3)
    qp_h = q_full.reshape(128, NG * 128).astype(np.float16)

    # d0[32s+n, k*32+q] = aT[dl] for q in 1..15 else 0
    a_r = a_l.reshape(4, 128, N)                  # [s, k, n]
    d0_full = np.zeros((4, 32, 128, 32))
    d0_full[:, :N, :, 1:16] = a_r.transpose(0, 2, 1)[:, :, :, None]
    d0_h = d0_full.reshape(128, NG * 32).astype(np.float16)

    return s_t_h, wstate_h, qp_h, d0_h


_NC_CACHE = None
LAST_RESULTS = None  # BassKernelResults of the most recent run (for test harness)


def _get_nc():
    global _NC_CACHE
    if _NC_CACHE is None:
        _NC_CACHE = build_nc()
    return _NC_CACHE


def prepare_in_maps(x, W_in, W_out, A_log, B_ssm, C_ssm, dt_log, D_ssm):
    x = np.asarray(x)
    W_in = np.asarray(W_in)
    W_out = np.asarray(W_out)

    S_T, W_state, Qp, aT = _ssm_operators(
        np.asarray(A_log), np.asarray(B_ssm), np.asarray(C_ssm),
        np.asarray(dt_log), np.asarray(D_ssm))

    half = [_half_arrays(S_T, W_state, Qp, aT, h) for h in range(2)]

    # per-half projection weights in device layout
    win_h, wout_h = [], []
    for h in range(2):
        Wl = W_in[h * DL:(h + 1) * DL, :]                      # [512, 1024]
        win_h.append(np.ascontiguousarray(
            Wl.T.reshape(8, 128, DL).transpose(1, 0, 2).reshape(128, 8 * DL)
        ).astype(np.float16))
        Wo = W_out[:, h * DL:(h + 1) * DL]                     # [1024, 512]
        wout_h.append(np.ascontiguousarray(
            Wo.T.reshape(4, 128, 1024).transpose(1, 0, 2).reshape(128, 4 * 1024)
        ).astype(np.float16))

    xT_b = []
    for b in range(B):
        xt = x[b].T                                            # [1024, 2048]
        xT_b.append(np.ascontiguousarray(
            xt.reshape(8, 128, L).transpose(1, 0, 2).reshape(128, 8 * L)
        ).astype(np.float16))

    in_maps = []
    for core in range(8):
        b, h = core // 2, core % 2
        s_t_h, wstate_h, qp_h, d0_h = half[h]
        in_maps.append({
            "xT": xT_b[b], "winT": win_h[h], "woutT": wout_h[h],
            "s_t": s_t_h, "wstate": wstate_h, "qp": qp_h, "d0": d0_h,
        })
    return in_maps


def run_device(in_maps):
    nc = _get_nc()
    res = run_bass_kernel_spmd(nc, in_maps, core_ids=list(range(8)))
    global LAST_RESULTS
    LAST_RESULTS = res
    return res


def gather_output(res):
    out = np.empty((B, L, D), dtype=np.float32)
    for b in range(B):
        acc = None
        for h in range(2):
            o = res.results[2 * b + h]["outT"].astype(np.float32)
            part = o.reshape(128, 8, L).transpose(1, 0, 2).reshape(D, L)
            acc = part if acc is None else acc + part
        out[b] = acc.T
    return out


def kernel(x, W_in, W_out, A_log, B_ssm, C_ssm, dt_log, D_ssm):
    in_maps = prepare_in_maps(x, W_in, W_out, A_log, B_ssm, C_ssm, dt_log, D_ssm)
    res = run_device(in_maps)
    return gather_output(res)
